# revision 3
# baseline (speedup 1.0000x reference)
"""Trainium2 Bass kernel for a dense transformer block (pre-LN, causal, RoPE).

Sharding: data-parallel over batch. B=8 batch elements, 8 NeuronCores, one
batch element per core; weights replicated. No collectives needed.

Per-core single-batch block (T=1024, D=1024, H=16, HS=64, F=4096), bf16
matmul inputs with f32 PSUM accumulation:
  LN1 (f32 stats; rstd = exp(-0.5*ln(var+eps)) keeps ACT on one table set)
  -> zn bf16 -> PE-transpose -> znT
  Q/K = znT @ wq/wk (hs-cols permuted even|odd, ln1_w folded) + RoPE (host
  cos/sin tables, HS**-0.5 q-scale folded) -> PE-transpose -> QT/KT [ch, T]
  V = znT @ wv -> Vslab [T, ch]
  per (head, q-block): scoresT[s,q] = kT_h.T @ qT_h; exp on ACT; causal
  0/1-mask multiply; attn@V with a col-tiled ones-matmul producing the
  softmax denominator in PSUM partitions 64:127; normalize via exp(-ln(d)).
  proj + residual, LN2, FFN (relu with per-partition bias on ACT),
  final residual, out f32.  All LN affine params and biases are folded
  host-side into the weight matrices / K=1 ones-row bias matmuls.
"""

import os
import sys
import numpy as np

for _p in ("/opt/trn_rl_repo", "/root/.axon_site/_ro/trn_rl_repo"):
    if os.path.isdir(_p) and _p not in sys.path:
        sys.path.append(_p)

import ml_dtypes

import concourse.bass as bass
import concourse.tile as tile
from concourse import bacc, mybir
from concourse.bass import ts
from concourse.bass_utils import run_bass_kernel_spmd

BF16 = mybir.dt.bfloat16
F32 = mybir.dt.float32
AF = mybir.ActivationFunctionType
ALU = mybir.AluOpType

B, T, D, H, HS, F = 8, 1024, 1024, 16, 64, 4096
NT = T // 128   # 8 T-tiles
ND = D // 128   # 8 D-chunks
NF = F // 128   # 32 F-chunks
NCORES = 8


def _bcast_heads(ap2d, nheads=H):
    """[128, J] AP -> [128, nheads, J] broadcast along a step-0 middle dim."""
    return bass.AP(ap2d.tensor, ap2d.offset, [ap2d.ap[0], [0, nheads], ap2d.ap[-1]])


def build_kernel():
    import contextlib

    nc = bacc.Bacc("TRN2", target_bir_lowering=False, debug=False,
                   num_devices=NCORES)

    # ---- external I/O ------------------------------------------------------
    xd = nc.dram_tensor("x", [T, D], F32, kind="ExternalInput")
    wq_d = nc.dram_tensor("wq", [128, ND, D], BF16, kind="ExternalInput")
    wk_d = nc.dram_tensor("wk", [128, ND, D], BF16, kind="ExternalInput")
    wv_d = nc.dram_tensor("wv", [128, ND, D], BF16, kind="ExternalInput")
    wp_d = nc.dram_tensor("wp", [128, ND, D], BF16, kind="ExternalInput")
    w1_d = nc.dram_tensor("w1", [ND, 128, F], BF16, kind="ExternalInput")
    w2_d = nc.dram_tensor("w2", [NF, 128, D], BF16, kind="ExternalInput")
    rope_d = nc.dram_tensor("rope", [128, NT, 4, HS], BF16, kind="ExternalInput")
    mask_d = nc.dram_tensor("mask", [128, 4, 512], BF16, kind="ExternalInput")
    ident_d = nc.dram_tensor("ident", [128, 128], BF16, kind="ExternalInput")
    ones64_d = nc.dram_tensor("ones64", [128, 64], BF16, kind="ExternalInput")
    onesrow_d = nc.dram_tensor("onesrow", [1, 128], BF16, kind="ExternalInput")
    brows_d = nc.dram_tensor("brows", [1, 4 * D], BF16, kind="ExternalInput")
    b1t_d = nc.dram_tensor("b1t", [128, NF], F32, kind="ExternalInput")
    out_d = nc.dram_tensor("out", [T, D], F32, kind="ExternalOutput")

    xr = xd.ap().rearrange("(a p) d -> p a d", p=128)       # [128, NT, D]
    outr = out_d.ap().rearrange("(a p) d -> p a d", p=128)  # [128, NT, D]

    with tile.TileContext(nc) as tc:
        ctx = contextlib.ExitStack()
        with ctx:
            consts = ctx.enter_context(tc.tile_pool(name="consts", bufs=1))
            slabs = ctx.enter_context(tc.tile_pool(name="slabs", bufs=4))
            xpool = ctx.enter_context(tc.tile_pool(name="xpool", bufs=1))
            small = ctx.enter_context(tc.tile_pool(name="small", bufs=3))
            ps_mm = ctx.enter_context(
                tc.tile_pool(name="ps_mm", bufs=4, space="PSUM"))
            ps_att = ctx.enter_context(
                tc.tile_pool(name="ps_att", bufs=4, space="PSUM"))

            # ---- global constants -----------------------------------------
            ident = consts.tile([128, 128], BF16)
            nc.sync.dma_start(out=ident, in_=ident_d.ap())
            ones64 = consts.tile([128, 64], BF16)
            nc.sync.dma_start(out=ones64, in_=ones64_d.ap())
            onesrow = consts.tile([1, 128], BF16)
            nc.sync.dma_start(out=onesrow, in_=onesrow_d.ap())
            brows = consts.tile([1, 4 * D], BF16)  # bq | bk | bproj | b2
            nc.sync.dma_start(out=brows, in_=brows_d.ap())
            b1t = consts.tile([128, NF], F32)
            nc.sync.dma_start(out=b1t, in_=b1t_d.ap())
            eps = consts.tile([128, 1], F32)
            nc.vector.memset(eps, 1e-5)

            # ---- x in ------------------------------------------------------
            x_sb = xpool.tile([128, NT, D], F32)
            nc.sync.dma_start(out=x_sb, in_=xr)

            # ---- layernorm (standardize only; affine folded host-side) -----
            def layernorm(src_sb, dst_bf16):
                """src [128, NT, D] -> dst [128, NT, D] bf16 standardized."""
                for tt in range(NT):
                    xt = src_sb[:, tt, :]
                    stats = small.tile([128, 2, 6], F32, tag="lnstats")
                    for g in range(2):
                        nc.vector.bn_stats(out=stats[:, g, :],
                                           in_=xt[:, g * 512:(g + 1) * 512])
                    mv = small.tile([128, 2], F32, tag="lnmv")
                    nc.vector.bn_aggr(out=mv, in_=stats)
                    # rstd = exp(-0.5 * ln(var + eps)); ln+exp share one ACT
                    # table set with the attention exp -> no table thrash.
                    std = small.tile([128, 2], F32, tag="lnstd")
                    nc.scalar.activation(out=std[:, 1:2], in_=mv[:, 1:2],
                                         func=AF.Ln, bias=eps, scale=1.0)
                    nc.scalar.activation(out=std[:, 0:1], in_=std[:, 1:2],
                                         func=AF.Exp, scale=-0.5)
                    nc.vector.tensor_scalar(
                        out=dst_bf16[:, tt, :], in0=xt,
                        scalar1=mv[:, 0:1], scalar2=std[:, 0:1],
                        op0=ALU.subtract, op1=ALU.mult)

            znT = slabs.tile([128, ND, T], BF16, tag="slab")
            zn = slabs.tile([128, NT, D], BF16, tag="slab")
            layernorm(x_sb, zn)
            for tt in range(NT):
                for c in range(ND):
                    pt = ps_att.tile([128, 128], BF16, tag="att")
                    nc.tensor.transpose(out=pt,
                                        in_=zn[:, tt, ts(c, 128)], identity=ident)
                    nc.scalar.activation(out=znT[:, c, ts(tt, 128)],
                                         in_=pt, func=AF.Copy)

            QT = slabs.tile([128, ND, T], BF16, tag="slab")
            KT = slabs.tile([128, ND, T], BF16, tag="slab")

            # ============ attention super-phase (scoped pool) ==============
            actx = contextlib.ExitStack()
            with actx:
                apool = actx.enter_context(tc.tile_pool(name="apool", bufs=2))
                ppool = actx.enter_context(tc.tile_pool(name="ppool", bufs=9))

                rope_sb = apool.tile([128, NT, 4, HS], BF16, tag="rope")
                nc.sync.dma_start(out=rope_sb, in_=rope_d.ap())
                mask_sb = apool.tile([128, 4, 512], BF16, tag="mask")
                nc.sync.dma_start(out=mask_sb, in_=mask_d.ap())

                def qkv_proj(w_dram, brow_idx):
                    w_sb = apool.tile([128, ND, D], BF16, tag="w")
                    nc.sync.dma_start(out=w_sb, in_=w_dram.ap())
                    for tt in range(NT):
                        ps0 = ps_mm.tile([128, 512], F32, tag="mm")
                        ps1 = ps_mm.tile([128, 512], F32, tag="mm")
                        last = ND - 1
                        for c in range(ND):
                            fin = (c == last and brow_idx is None)
                            lhsT = znT[:, c, ts(tt, 128)]
                            nc.tensor.matmul(ps0, lhsT, w_sb[:, c, 0:512],
                                             start=(c == 0), stop=fin)
                            nc.tensor.matmul(ps1, lhsT, w_sb[:, c, 512:1024],
                                             start=(c == 0), stop=fin)
                        if brow_idx is not None:
                            o = brow_idx * D
                            nc.tensor.matmul(ps0, onesrow,
                                             brows[0:1, o:o + 512],
                                             start=False, stop=True)
                            nc.tensor.matmul(ps1, onesrow,
                                             brows[0:1, o + 512:o + 1024],
                                             start=False, stop=True)
                        yield tt, ps0, ps1

                # -- Q then K: copy out of PSUM, rope, transpose
                for w_dram, brow_idx, dstT, tblc, tbls in (
                        (wq_d, 0, QT, 0, 1), (wk_d, 1, KT, 2, 3)):
                    for tt, ps0, ps1 in qkv_proj(w_dram, brow_idx):
                        raw = apool.tile([128, D], BF16, tag="qkraw")
                        nc.scalar.activation(out=raw[:, 0:512], in_=ps0,
                                             func=AF.Copy)
                        nc.scalar.activation(out=raw[:, 512:1024], in_=ps1,
                                             func=AF.Copy)
                        rot = apool.tile([128, D], BF16, tag="qkrot")
                        rv = rot.rearrange("p (h x j) -> p h x j", h=H, x=2)
                        qv = raw.rearrange("p (h x j) -> p h x j", h=H, x=2)
                        cos_t = _bcast_heads(rope_sb[:, tt, tblc, :])
                        cos_t = bass.AP(cos_t.tensor, cos_t.offset,
                                        cos_t.ap[:2] + [[32, 2], [1, 32]])
                        sin_e = _bcast_heads(rope_sb[:, tt, tbls, 0:32])
                        sin_o = _bcast_heads(rope_sb[:, tt, tbls, 32:64])
                        tmp = apool.tile([128, D], BF16, tag="qktmp")
                        tv = tmp.rearrange("p (h x j) -> p h x j", h=H, x=2)
                        # tmp = swap_halves(q) * (+-sin)
                        nc.vector.tensor_mul(out=tv[:, :, 0, :],
                                             in0=qv[:, :, 1, :], in1=sin_e)
                        nc.vector.tensor_mul(out=tv[:, :, 1, :],
                                             in0=qv[:, :, 0, :], in1=sin_o)
                        nc.vector.tensor_mul(out=rv, in0=qv, in1=cos_t)
                        nc.vector.tensor_add(out=rot, in0=rot, in1=tmp)
                        for c in range(ND):
                            pt = ps_att.tile([128, 128], BF16, tag="att")
                            nc.tensor.transpose(out=pt,
                                                in_=rot[:, ts(c, 128)],
                                                identity=ident)
                            nc.scalar.activation(out=dstT[:, c, ts(tt, 128)],
                                                 in_=pt, func=AF.Copy)

                # -- V (plain copy; ln1_b contribution folded into b_proj)
                Vs = slabs.tile([128, NT, D], BF16, tag="slab")
                for tt, ps0, ps1 in qkv_proj(wv_d, None):
                    nc.scalar.activation(out=Vs[:, tt, 0:512], in_=ps0,
                                         func=AF.Copy)
                    nc.scalar.activation(out=Vs[:, tt, 512:1024], in_=ps1,
                                         func=AF.Copy)

                # -- attention
                oT = slabs.tile([128, ND, T], BF16, tag="slab")
                for qb in range(2):
                    n_sc = 4 * (qb + 1)
                    for h in range(H):
                        prow = (h % 2) * 64
                        cidx = h // 2
                        qsl = slice(qb * 512, (qb + 1) * 512)
                        kT_h = KT[prow:prow + 64, cidx, :]
                        qT_h = QT[prow:prow + 64, cidx, qsl]
                        ptiles = []
                        for sc in range(n_sc):
                            ps = ps_att.tile([128, 512], F32, tag="att")
                            nc.tensor.matmul(ps, kT_h[:, ts(sc, 128)], qT_h,
                                             start=True, stop=True)
                            P = ppool.tile([128, 512], BF16, tag="P")
                            nc.scalar.activation(out=P, in_=ps, func=AF.Exp)
                            j = sc - 4 * qb
                            if j >= 0:
                                nc.vector.tensor_mul(out=P, in0=P,
                                                     in1=mask_sb[:, j, :])
                            ptiles.append(P)
                        po = ps_att.tile([128, 512], F32, tag="att")
                        for sc in range(n_sc):
                            st, sp = (sc == 0), (sc == n_sc - 1)
                            nc.tensor.matmul(po[0:64, :],
                                             Vs[:, sc, h * 64:(h + 1) * 64],
                                             ptiles[sc], start=st, stop=sp,
                                             tile_position=(0, 0))
                            nc.tensor.matmul(po[64:128, :], ones64, ptiles[sc],
                                             start=st, stop=sp,
                                             tile_position=(0, 64))
                        lnd = small.tile([64, 512], F32, tag="lnd")
                        nc.scalar.activation(out=lnd, in_=po[64:128, :],
                                             func=AF.Ln)
                        rec = small.tile([64, 512], BF16, tag="rec")
                        nc.scalar.activation(out=rec, in_=lnd, func=AF.Exp,
                                             scale=-1.0)
                        nc.vector.tensor_mul(out=oT[prow:prow + 64, cidx, qsl],
                                             in0=po[0:64, :], in1=rec)

                # -- proj + residual -> x2 (bf16)
                wp_sb = apool.tile([128, ND, D], BF16, tag="w")
                nc.sync.dma_start(out=wp_sb, in_=wp_d.ap())
                x2 = slabs.tile([128, NT, D], BF16, tag="slab")
                for tt in range(NT):
                    ps0 = ps_mm.tile([128, 512], F32, tag="mm")
                    ps1 = ps_mm.tile([128, 512], F32, tag="mm")
                    for c in range(ND):
                        lhsT = oT[:, c, ts(tt, 128)]
                        nc.tensor.matmul(ps0, lhsT, wp_sb[:, c, 0:512],
                                         start=(c == 0), stop=False)
                        nc.tensor.matmul(ps1, lhsT, wp_sb[:, c, 512:1024],
                                         start=(c == 0), stop=False)
                    nc.tensor.matmul(ps0, onesrow, brows[0:1, 2 * D:2 * D + 512],
                                     start=False, stop=True)
                    nc.tensor.matmul(ps1, onesrow,
                                     brows[0:1, 2 * D + 512:3 * D],
                                     start=False, stop=True)
                    nc.vector.tensor_add(out=x2[:, tt, 0:512], in0=ps0,
                                         in1=x_sb[:, tt, 0:512])
                    nc.vector.tensor_add(out=x2[:, tt, 512:1024], in0=ps1,
                                         in1=x_sb[:, tt, 512:1024])

            # ---- LN2 + transpose ------------------------------------------
            z2 = slabs.tile([128, NT, D], BF16, tag="slab")
            z2T = slabs.tile([128, ND, T], BF16, tag="slab")
            layernorm(x2, z2)
            for tt in range(NT):
                for c in range(ND):
                    pt = ps_att.tile([128, 128], BF16, tag="att")
                    nc.tensor.transpose(out=pt,
                                        in_=z2[:, tt, ts(c, 128)], identity=ident)
                    nc.scalar.activation(out=z2T[:, c, ts(tt, 128)],
                                         in_=pt, func=AF.Copy)

            # ============ FFN super-phase (scoped pool) ====================
            fctx = contextlib.ExitStack()
            with fctx:
                fpool = fctx.enter_context(tc.tile_pool(name="fpool", bufs=1))
                w1pool = fctx.enter_context(tc.tile_pool(name="w1pool", bufs=2))
                opool = fctx.enter_context(tc.tile_pool(name="opool", bufs=4))
                for tb in range(2):
                    tbs = slice(tb * 512, (tb + 1) * 512)
                    # FFN1 half: hT[f, t-half] = relu(w1.T @ z2T + b1)
                    hTh = fpool.tile([128, NF, 512], BF16, tag="hTh")
                    for mg in range(NF // 4):
                        w1g = w1pool.tile([128, ND, 512], BF16, tag="w1g")
                        nc.sync.dma_start(
                            out=w1g,
                            in_=w1_d.ap()[:, :, mg * 512:(mg + 1) * 512]
                            .rearrange("c p f -> p c f"))
                        for mi in range(4):
                            m = mg * 4 + mi
                            ps = ps_mm.tile([128, 512], F32, tag="mm")
                            for c in range(ND):
                                nc.tensor.matmul(
                                    ps, w1g[:, c, ts(mi, 128)],
                                    z2T[:, c, tbs],
                                    start=(c == 0), stop=(c == ND - 1))
                            nc.scalar.activation(
                                out=hTh[:, m, :], in_=ps,
                                func=AF.Relu, bias=b1t[:, m:m + 1], scale=1.0)
                    # FFN2 half + residual -> out
                    for db in range(2):
                        w2h = fpool.tile([128, NF, 512], BF16, tag="w2h")
                        nc.sync.dma_start(
                            out=w2h,
                            in_=w2_d.ap()[:, :, db * 512:(db + 1) * 512]
                            .rearrange("c p f -> p c f"))
                        for tl in range(4):
                            tt = tb * 4 + tl
                            ps = ps_mm.tile([128, 512], F32, tag="mm")
                            for c in range(NF):
                                nc.tensor.matmul(ps, hTh[:, c, ts(tl, 128)],
                                                 w2h[:, c, :],
                                                 start=(c == 0), stop=False)
                            o = 3 * D + db * 512
                            nc.tensor.matmul(ps, onesrow, brows[0:1, o:o + 512],
                                             start=False, stop=True)
                            ot = opool.tile([128, 512], F32, tag="ot")
                            nc.vector.tensor_add(
                                out=ot, in0=ps,
                                in1=x2[:, tt, db * 512:(db + 1) * 512])
                            nc.sync.dma_start(
                                out=outr[:, tt, db * 512:(db + 1) * 512],
                                in_=ot)

    nc.compile()
    return nc


def _prep_inputs(inputs):
    """Host-side preprocessing: fold LN affine, permute rope cols, cast bf16."""
    f32 = np.float32
    x = np.asarray(inputs["x"], f32)
    wq = np.asarray(inputs["wq"], f32)
    wk = np.asarray(inputs["wk"], f32)
    wv = np.asarray(inputs["wv"], f32)
    w_proj = np.asarray(inputs["w_proj"], f32)
    b_proj = np.asarray(inputs["b_proj"], f32)
    ln1_w = np.asarray(inputs["ln1_w"], f32)
    ln1_b = np.asarray(inputs["ln1_b"], f32)
    ln2_w = np.asarray(inputs["ln2_w"], f32)
    ln2_b = np.asarray(inputs["ln2_b"], f32)
    w1 = np.asarray(inputs["w1"], f32)
    b1 = np.asarray(inputs["b1"], f32)
    w2 = np.asarray(inputs["w2"], f32)
    b2 = np.asarray(inputs["b2"], f32)

    bf = ml_dtypes.bfloat16
    perm = np.concatenate([np.arange(0, HS, 2), np.arange(1, HS, 2)])
    idx = (np.arange(H)[:, None] * HS + perm[None, :]).reshape(-1)

    wq_flat = wq.transpose(1, 0, 2).reshape(D, H * HS)
    wk_flat = wk.transpose(1, 0, 2).reshape(D, H * HS)
    wv_flat = wv.transpose(1, 0, 2).reshape(D, H * HS)
    wq_p = wq_flat[:, idx]
    wk_p = wk_flat[:, idx]

    def wlayout(w):  # [D, D] -> [128, ND, D]  (p = d_in, c = d_chunk)
        return np.ascontiguousarray(
            w.reshape(ND, 128, D).transpose(1, 0, 2)).astype(bf)

    wq_h = wlayout(ln1_w[:, None] * wq_p)
    wk_h = wlayout(ln1_w[:, None] * wk_p)
    wv_h = wlayout(ln1_w[:, None] * wv_flat)
    wp_h = wlayout(w_proj)
    w1_h = np.ascontiguousarray(
        (ln2_w[:, None] * w1).reshape(ND, 128, F)).astype(bf)
    w2_h = np.ascontiguousarray(w2.reshape(NF, 128, D)).astype(bf)

    bq = ln1_b @ wq_p
    bk = ln1_b @ wk_p
    bv = ln1_b @ wv_flat
    bproj_eff = b_proj + bv @ w_proj
    b1_eff = ln2_b @ w1 + b1
    brows = np.concatenate([bq, bk, bproj_eff, b2]).reshape(1, 4 * D).astype(bf)
    b1t = np.ascontiguousarray(b1_eff.reshape(NF, 128).T).astype(f32)

    # rope tables: [128, NT, 4, HS]; 4 = (cos_q, sin_q, cos_k, sin_k)
    t = np.arange(T, dtype=f32)
    th = (1.0 / 10000.0 ** (np.arange(0, HS, 2, dtype=f32) / f32(HS))).astype(f32)
    ang = t[:, None] * th[None, :]
    cos = np.concatenate([np.cos(ang), np.cos(ang)], 1)           # [T, HS]
    sin = np.concatenate([-np.sin(ang), np.sin(ang)], 1)
    sc = f32(HS) ** f32(-0.5)
    rope = np.stack([cos * sc, sin * sc, cos, sin], 1)            # [T, 4, HS]
    rope_h = np.ascontiguousarray(
        rope.reshape(NT, 128, 4, HS).transpose(1, 0, 2, 3)).astype(bf)

    # causal 0/1 masks for the 4 diagonal-crossing s-tiles of a 512 q-block
    sl = np.arange(128)[:, None]
    ql = np.arange(512)[None, :]
    mask = np.stack([(j * 128 + sl <= ql) for j in range(4)]).astype(bf)
    mask_h = np.ascontiguousarray(mask.transpose(1, 0, 2))        # [128, 4, 512]

    common = {
        "wq": wq_h, "wk": wk_h, "wv": wv_h, "wp": wp_h,
        "w1": w1_h, "w2": w2_h,
        "rope": rope_h, "mask": mask_h,
        "ident": np.eye(128, dtype=bf),
        "ones64": np.ones((128, 64), bf),
        "onesrow": np.ones((1, 128), bf),
        "brows": brows, "b1t": b1t,
    }
    in_maps = [dict(common, x=np.ascontiguousarray(x[b])) for b in range(B)]
    return in_maps


_NC_CACHE = {}


def get_nc():
    if "nc" not in _NC_CACHE:
        _NC_CACHE["nc"] = build_kernel()
    return _NC_CACHE["nc"]


def kernel(**inputs):
    nc = get_nc()
    in_maps = _prep_inputs(inputs)
    res = run_bass_kernel_spmd(nc, in_maps, core_ids=list(range(NCORES)))
    out = np.stack([res.results[i]["out"] for i in range(NCORES)])
    return out.astype(np.float32)


# revision 4
# speedup vs baseline: 1.1728x; 1.1728x over previous
"""Trainium2 Bass kernel for a dense transformer block (pre-LN, causal, RoPE).

Sharding: data-parallel over batch. B=8 batch elements, 8 NeuronCores, one
batch element per core; weights replicated. No collectives needed.

Per-core single-batch block (T=1024, D=1024, H=16, HS=64, F=4096), bf16
matmul inputs with f32 PSUM accumulation:
  LN1 (f32 stats; rstd = exp(-0.5*ln(var+eps)) keeps ACT on one table set)
  -> zn bf16 -> DMA-transpose -> znT
  Q/K = znT @ wq/wk (hs-cols permuted even|odd, ln1_w folded) + RoPE (host
  cos/sin tables, HS**-0.5 q-scale folded) -> DMA-transpose -> QT/KT [ch, T]
  V = znT @ wv -> Vslab [T, ch]
  attention processed in head PAIRS (even head -> PE cols 0:63, odd head ->
  cols 64:127 via tile_position col-tiling): scoresT[s,q] pairs share one
  2-bank PSUM tile so exp runs once per [128,1024]; causal 0/1 pair-mask
  multiply; attn@V writes o_h|o_h' into one PSUM tile and a ones-matmul
  pair writes the softmax denominators into a second tile; normalize via
  exp(-ln(d)) on ACT + one DVE multiply per head-pair.
  proj + residual, LN2, FFN (relu+bias as one DVE tensor_scalar add/max),
  final residual, out f32.  All LN affine params and biases are folded
  host-side into the weight matrices / K=1 ones-row bias matmuls.
"""

import os
import sys
import numpy as np

for _p in ("/opt/trn_rl_repo", "/root/.axon_site/_ro/trn_rl_repo"):
    if os.path.isdir(_p) and _p not in sys.path:
        sys.path.append(_p)

import ml_dtypes

import concourse.bass as bass
import concourse.tile as tile
from concourse import bacc, mybir
from concourse.bass import ts
from concourse.bass_utils import run_bass_kernel_spmd

BF16 = mybir.dt.bfloat16
F32 = mybir.dt.float32
AF = mybir.ActivationFunctionType
ALU = mybir.AluOpType

B, T, D, H, HS, F = 8, 1024, 1024, 16, 64, 4096
NT = T // 128   # 8 T-tiles
ND = D // 128   # 8 D-chunks
NF = F // 128   # 32 F-chunks
NCORES = 8


def _bcast_heads(ap2d, nheads=H):
    """[128, J] AP -> [128, nheads, J] broadcast along a step-0 middle dim."""
    return bass.AP(ap2d.tensor, ap2d.offset, [ap2d.ap[0], [0, nheads], ap2d.ap[-1]])


def build_kernel():
    import contextlib

    nc = bacc.Bacc("TRN2", target_bir_lowering=False, debug=False,
                   num_devices=NCORES)

    # ---- external I/O ------------------------------------------------------
    xd = nc.dram_tensor("x", [T, D], F32, kind="ExternalInput")
    wq_d = nc.dram_tensor("wq", [128, ND, D], BF16, kind="ExternalInput")
    wk_d = nc.dram_tensor("wk", [128, ND, D], BF16, kind="ExternalInput")
    wv_d = nc.dram_tensor("wv", [128, ND, D], BF16, kind="ExternalInput")
    wp_d = nc.dram_tensor("wp", [128, ND, D], BF16, kind="ExternalInput")
    w1_d = nc.dram_tensor("w1", [ND, 128, F], BF16, kind="ExternalInput")
    w2_d = nc.dram_tensor("w2", [NF, 128, D], BF16, kind="ExternalInput")
    rope_d = nc.dram_tensor("rope", [128, NT, 4, HS], BF16, kind="ExternalInput")
    mask_d = nc.dram_tensor("mask", [128, 2, 1024], BF16, kind="ExternalInput")
    ones64_d = nc.dram_tensor("ones64", [128, 64], BF16, kind="ExternalInput")
    onesrow_d = nc.dram_tensor("onesrow", [1, 128], BF16, kind="ExternalInput")
    brows_d = nc.dram_tensor("brows", [1, 4 * D], BF16, kind="ExternalInput")
    b1t_d = nc.dram_tensor("b1t", [128, NF], F32, kind="ExternalInput")
    out_d = nc.dram_tensor("out", [T, D], F32, kind="ExternalOutput")

    xr = xd.ap().rearrange("(a p) d -> p a d", p=128)       # [128, NT, D]
    outr = out_d.ap().rearrange("(a p) d -> p a d", p=128)  # [128, NT, D]

    with tile.TileContext(nc) as tc:
        ctx = contextlib.ExitStack()
        with ctx:
            consts = ctx.enter_context(tc.tile_pool(name="consts", bufs=1))
            slabs = ctx.enter_context(tc.tile_pool(name="slabs", bufs=4))
            xpool = ctx.enter_context(tc.tile_pool(name="xpool", bufs=1))
            small = ctx.enter_context(tc.tile_pool(name="small", bufs=3))
            psA = ctx.enter_context(  # 2-bank tiles: QKV/proj/score-pairs
                tc.tile_pool(name="psA", bufs=2, space="PSUM"))
            psB = ctx.enter_context(  # 1-bank tiles: attnV, FFN
                tc.tile_pool(name="psB", bufs=4, space="PSUM"))

            # ---- global constants -----------------------------------------
            ones64 = consts.tile([128, 64], BF16)
            nc.sync.dma_start(out=ones64, in_=ones64_d.ap())
            onesrow = consts.tile([1, 128], BF16)
            nc.sync.dma_start(out=onesrow, in_=onesrow_d.ap())
            brows = consts.tile([1, 4 * D], BF16)  # bq | bk | bproj | b2
            nc.sync.dma_start(out=brows, in_=brows_d.ap())
            b1t = consts.tile([128, NF], F32)
            nc.sync.dma_start(out=b1t, in_=b1t_d.ap())
            eps = consts.tile([128, 1], F32)
            nc.vector.memset(eps, 1e-5)

            # ---- x in ------------------------------------------------------
            x_sb = xpool.tile([128, NT, D], F32)
            nc.sync.dma_start(out=x_sb, in_=xr)

            # ---- layernorm (standardize only; affine folded host-side) -----
            def layernorm(src_sb, dst_bf16):
                """src [128, NT, D] -> dst [128, NT, D] bf16 standardized."""
                for tt in range(NT):
                    xt = src_sb[:, tt, :]
                    stats = small.tile([128, 2, 6], F32, tag="lnstats")
                    for g in range(2):
                        nc.vector.bn_stats(out=stats[:, g, :],
                                           in_=xt[:, g * 512:(g + 1) * 512])
                    mv = small.tile([128, 2], F32, tag="lnmv")
                    nc.vector.bn_aggr(out=mv, in_=stats)
                    # rstd = exp(-0.5 * ln(var + eps)); ln+exp share one ACT
                    # table set with the attention exp -> no table thrash.
                    std = small.tile([128, 2], F32, tag="lnstd")
                    nc.scalar.activation(out=std[:, 1:2], in_=mv[:, 1:2],
                                         func=AF.Ln, bias=eps, scale=1.0)
                    nc.scalar.activation(out=std[:, 0:1], in_=std[:, 1:2],
                                         func=AF.Exp, scale=-0.5)
                    nc.vector.tensor_scalar(
                        out=dst_bf16[:, tt, :], in0=xt,
                        scalar1=mv[:, 0:1], scalar2=std[:, 0:1],
                        op0=ALU.subtract, op1=ALU.mult)

            znT = slabs.tile([128, ND, T], BF16, tag="slab")
            zn = slabs.tile([128, NT, D], BF16, tag="slab")
            layernorm(x_sb, zn)
            for tt in range(NT):
                nc.sync.dma_start(out=znT[:, :, ts(tt, 128)],
                                  in_=zn[:, tt, :], transpose=True)

            QT = slabs.tile([128, ND, T], BF16, tag="slab")
            KT = slabs.tile([128, ND, T], BF16, tag="slab")

            # ============ attention super-phase (scoped pool) ==============
            actx = contextlib.ExitStack()
            with actx:
                apool = actx.enter_context(tc.tile_pool(name="apool", bufs=2))
                ppool = actx.enter_context(tc.tile_pool(name="ppool", bufs=9))

                rope_sb = apool.tile([128, NT, 4, HS], BF16, tag="rope")
                nc.sync.dma_start(out=rope_sb, in_=rope_d.ap())
                mask_sb = apool.tile([128, 2, 1024], BF16, tag="mask")
                nc.sync.dma_start(out=mask_sb, in_=mask_d.ap())

                def qkv_proj(w_dram, brow_idx):
                    w_sb = apool.tile([128, ND, D], BF16, tag="w")
                    nc.sync.dma_start(out=w_sb, in_=w_dram.ap())
                    for tt in range(NT):
                        ps = psA.tile([128, 1024], F32, tag="A")
                        last = ND - 1
                        for c in range(ND):
                            fin = (c == last and brow_idx is None)
                            lhsT = znT[:, c, ts(tt, 128)]
                            nc.tensor.matmul(ps[:, 0:512], lhsT,
                                             w_sb[:, c, 0:512],
                                             start=(c == 0), stop=fin)
                            nc.tensor.matmul(ps[:, 512:1024], lhsT,
                                             w_sb[:, c, 512:1024],
                                             start=(c == 0), stop=fin)
                        if brow_idx is not None:
                            o = brow_idx * D
                            nc.tensor.matmul(ps[:, 0:512], onesrow,
                                             brows[0:1, o:o + 512],
                                             start=False, stop=True)
                            nc.tensor.matmul(ps[:, 512:1024], onesrow,
                                             brows[0:1, o + 512:o + 1024],
                                             start=False, stop=True)
                        yield tt, ps

                # -- Q then K: copy out of PSUM, rope, DMA-transpose
                for w_dram, brow_idx, dstT, tblc, tbls in (
                        (wq_d, 0, QT, 0, 1), (wk_d, 1, KT, 2, 3)):
                    for tt, ps in qkv_proj(w_dram, brow_idx):
                        raw = apool.tile([128, D], BF16, tag="qkraw")
                        nc.scalar.activation(out=raw, in_=ps, func=AF.Copy)
                        rot = apool.tile([128, D], BF16, tag="qkrot")
                        rv = rot.rearrange("p (h x j) -> p h x j", h=H, x=2)
                        qv = raw.rearrange("p (h x j) -> p h x j", h=H, x=2)
                        cos_t = _bcast_heads(rope_sb[:, tt, tblc, :])
                        cos_t = bass.AP(cos_t.tensor, cos_t.offset,
                                        cos_t.ap[:2] + [[32, 2], [1, 32]])
                        sin_e = _bcast_heads(rope_sb[:, tt, tbls, 0:32])
                        sin_o = _bcast_heads(rope_sb[:, tt, tbls, 32:64])
                        tmp = apool.tile([128, D], BF16, tag="qktmp")
                        tv = tmp.rearrange("p (h x j) -> p h x j", h=H, x=2)
                        # tmp = swap_halves(q) * (+-sin)
                        nc.vector.tensor_mul(out=tv[:, :, 0, :],
                                             in0=qv[:, :, 1, :], in1=sin_e)
                        nc.vector.tensor_mul(out=tv[:, :, 1, :],
                                             in0=qv[:, :, 0, :], in1=sin_o)
                        nc.vector.tensor_mul(out=rv, in0=qv, in1=cos_t)
                        nc.vector.tensor_add(out=rot, in0=rot, in1=tmp)
                        nc.sync.dma_start(out=dstT[:, :, ts(tt, 128)],
                                          in_=rot, transpose=True)

                # -- V (plain copy; ln1_b contribution folded into b_proj)
                Vs = slabs.tile([128, NT, D], BF16, tag="slab")
                for tt, ps in qkv_proj(wv_d, None):
                    nc.scalar.activation(out=Vs[:, tt, :], in_=ps, func=AF.Copy)

                # -- attention, head pairs (h even -> cols 0:63, h+1 -> 64:127)
                oT = slabs.tile([128, ND, T], BF16, tag="slab")
                for qb in range(2):
                    n_sc = 4 * (qb + 1)
                    qsl = slice(qb * 512, (qb + 1) * 512)
                    for cidx in range(ND):          # head pair (2*cidx, 2*cidx+1)
                        kT0 = KT[0:64, cidx, :]
                        kT1 = KT[64:128, cidx, :]
                        qT0 = QT[0:64, cidx, qsl]
                        qT1 = QT[64:128, cidx, qsl]
                        pt0, pt1 = [], []
                        for spr in range(n_sc // 2):
                            sc0, sc1 = 2 * spr, 2 * spr + 1
                            for kT_h, qT_h, plist in ((kT0, qT0, pt0),
                                                      (kT1, qT1, pt1)):
                                ps = psA.tile([128, 1024], F32, tag="A")
                                nc.tensor.matmul(ps[:, 0:512],
                                                 kT_h[:, ts(sc0, 128)], qT_h,
                                                 start=True, stop=True)
                                nc.tensor.matmul(ps[:, 512:1024],
                                                 kT_h[:, ts(sc1, 128)], qT_h,
                                                 start=True, stop=True)
                                P = ppool.tile([128, 1024], BF16, tag="P")
                                nc.scalar.activation(out=P, in_=ps, func=AF.Exp)
                                if spr >= 2 * qb:   # diagonal-crossing pair
                                    nc.vector.tensor_mul(
                                        out=P, in0=P,
                                        in1=mask_sb[:, spr - 2 * qb, :])
                                plist.append(P)
                        po = psB.tile([128, 512], F32, tag="B")
                        dp = psB.tile([128, 512], F32, tag="B")
                        h0 = 2 * cidx
                        for sc in range(n_sc):
                            st, sp = (sc == 0), (sc == n_sc - 1)
                            P0 = pt0[sc // 2][:, (sc % 2) * 512:(sc % 2) * 512 + 512]
                            P1 = pt1[sc // 2][:, (sc % 2) * 512:(sc % 2) * 512 + 512]
                            nc.tensor.matmul(po[0:64, :],
                                             Vs[:, sc, h0 * 64:h0 * 64 + 64],
                                             P0, start=st, stop=sp,
                                             tile_position=(0, 0))
                            nc.tensor.matmul(po[64:128, :],
                                             Vs[:, sc, h0 * 64 + 64:h0 * 64 + 128],
                                             P1, start=st, stop=sp,
                                             tile_position=(0, 64))
                            nc.tensor.matmul(dp[0:64, :], ones64, P0,
                                             start=st, stop=sp,
                                             tile_position=(0, 0))
                            nc.tensor.matmul(dp[64:128, :], ones64, P1,
                                             start=st, stop=sp,
                                             tile_position=(0, 64))
                        lnd = apool.tile([128, 512], F32, tag="lnd")
                        nc.scalar.activation(out=lnd, in_=dp, func=AF.Ln)
                        rec = apool.tile([128, 512], BF16, tag="rec")
                        nc.scalar.activation(out=rec, in_=lnd, func=AF.Exp,
                                             scale=-1.0)
                        nc.vector.tensor_mul(out=oT[:, cidx, qsl],
                                             in0=po, in1=rec)

                # -- proj + residual -> x2 (bf16)
                wp_sb = apool.tile([128, ND, D], BF16, tag="w")
                nc.sync.dma_start(out=wp_sb, in_=wp_d.ap())
                x2 = slabs.tile([128, NT, D], BF16, tag="slab")
                for tt in range(NT):
                    ps = psA.tile([128, 1024], F32, tag="A")
                    for c in range(ND):
                        lhsT = oT[:, c, ts(tt, 128)]
                        nc.tensor.matmul(ps[:, 0:512], lhsT,
                                         wp_sb[:, c, 0:512],
                                         start=(c == 0), stop=False)
                        nc.tensor.matmul(ps[:, 512:1024], lhsT,
                                         wp_sb[:, c, 512:1024],
                                         start=(c == 0), stop=False)
                    nc.tensor.matmul(ps[:, 0:512], onesrow,
                                     brows[0:1, 2 * D:2 * D + 512],
                                     start=False, stop=True)
                    nc.tensor.matmul(ps[:, 512:1024], onesrow,
                                     brows[0:1, 2 * D + 512:3 * D],
                                     start=False, stop=True)
                    nc.vector.tensor_add(out=x2[:, tt, :], in0=ps,
                                         in1=x_sb[:, tt, :])

            # ---- LN2 + DMA-transpose --------------------------------------
            z2 = slabs.tile([128, NT, D], BF16, tag="slab")
            z2T = slabs.tile([128, ND, T], BF16, tag="slab")
            layernorm(x2, z2)
            for tt in range(NT):
                nc.sync.dma_start(out=z2T[:, :, ts(tt, 128)],
                                  in_=z2[:, tt, :], transpose=True)

            # ============ FFN super-phase (scoped pool) ====================
            fctx = contextlib.ExitStack()
            with fctx:
                fpool = fctx.enter_context(tc.tile_pool(name="fpool", bufs=1))
                w1pool = fctx.enter_context(tc.tile_pool(name="w1pool", bufs=2))
                w2pool = fctx.enter_context(tc.tile_pool(name="w2pool", bufs=5))
                opool = fctx.enter_context(tc.tile_pool(name="opool", bufs=4))
                for tb in range(2):
                    tbs = slice(tb * 512, (tb + 1) * 512)
                    # FFN1 half: hT[f, t-half] = relu(w1.T @ z2T + b1) on DVE
                    hTh = fpool.tile([128, NF, 512], BF16, tag="hTh")
                    for mg in range(NF // 4):
                        w1g = w1pool.tile([128, ND, 512], BF16, tag="w1g")
                        nc.sync.dma_start(
                            out=w1g,
                            in_=w1_d.ap()[:, :, mg * 512:(mg + 1) * 512]
                            .rearrange("c p f -> p c f"))
                        for mi in range(4):
                            m = mg * 4 + mi
                            ps = psB.tile([128, 512], F32, tag="B")
                            for c in range(ND):
                                nc.tensor.matmul(
                                    ps, w1g[:, c, ts(mi, 128)],
                                    z2T[:, c, tbs],
                                    start=(c == 0), stop=(c == ND - 1))
                            nc.vector.tensor_scalar(
                                out=hTh[:, m, :], in0=ps,
                                scalar1=b1t[:, m:m + 1], scalar2=0.0,
                                op0=ALU.add, op1=ALU.max)
                    # FFN2 half + residual -> out
                    for db in range(2):
                        w2cs = []
                        for cg in range(4):
                            w2c = w2pool.tile([128, 8, 512], BF16, tag="w2c")
                            nc.sync.dma_start(
                                out=w2c,
                                in_=w2_d.ap()[cg * 8:(cg + 1) * 8, :,
                                              db * 512:(db + 1) * 512]
                                .rearrange("c p f -> p c f"))
                            w2cs.append(w2c)
                        for tl in range(4):
                            tt = tb * 4 + tl
                            ps = psB.tile([128, 512], F32, tag="B")
                            for c in range(NF):
                                nc.tensor.matmul(ps, hTh[:, c, ts(tl, 128)],
                                                 w2cs[c // 8][:, c % 8, :],
                                                 start=(c == 0), stop=False)
                            o = 3 * D + db * 512
                            nc.tensor.matmul(ps, onesrow, brows[0:1, o:o + 512],
                                             start=False, stop=True)
                            ot = opool.tile([128, 512], F32, tag="ot")
                            nc.vector.tensor_add(
                                out=ot, in0=ps,
                                in1=x2[:, tt, db * 512:(db + 1) * 512])
                            nc.sync.dma_start(
                                out=outr[:, tt, db * 512:(db + 1) * 512],
                                in_=ot)

    nc.compile()
    return nc


def _prep_inputs(inputs):
    """Host-side preprocessing: fold LN affine, permute rope cols, cast bf16."""
    f32 = np.float32
    x = np.asarray(inputs["x"], f32)
    wq = np.asarray(inputs["wq"], f32)
    wk = np.asarray(inputs["wk"], f32)
    wv = np.asarray(inputs["wv"], f32)
    w_proj = np.asarray(inputs["w_proj"], f32)
    b_proj = np.asarray(inputs["b_proj"], f32)
    ln1_w = np.asarray(inputs["ln1_w"], f32)
    ln1_b = np.asarray(inputs["ln1_b"], f32)
    ln2_w = np.asarray(inputs["ln2_w"], f32)
    ln2_b = np.asarray(inputs["ln2_b"], f32)
    w1 = np.asarray(inputs["w1"], f32)
    b1 = np.asarray(inputs["b1"], f32)
    w2 = np.asarray(inputs["w2"], f32)
    b2 = np.asarray(inputs["b2"], f32)

    bf = ml_dtypes.bfloat16
    perm = np.concatenate([np.arange(0, HS, 2), np.arange(1, HS, 2)])
    idx = (np.arange(H)[:, None] * HS + perm[None, :]).reshape(-1)

    wq_flat = wq.transpose(1, 0, 2).reshape(D, H * HS)
    wk_flat = wk.transpose(1, 0, 2).reshape(D, H * HS)
    wv_flat = wv.transpose(1, 0, 2).reshape(D, H * HS)
    wq_p = wq_flat[:, idx]
    wk_p = wk_flat[:, idx]

    def wlayout(w):  # [D, D] -> [128, ND, D]  (p = d_in, c = d_chunk)
        return np.ascontiguousarray(
            w.reshape(ND, 128, D).transpose(1, 0, 2)).astype(bf)

    wq_h = wlayout(ln1_w[:, None] * wq_p)
    wk_h = wlayout(ln1_w[:, None] * wk_p)
    wv_h = wlayout(ln1_w[:, None] * wv_flat)
    wp_h = wlayout(w_proj)
    w1_h = np.ascontiguousarray(
        (ln2_w[:, None] * w1).reshape(ND, 128, F)).astype(bf)
    w2_h = np.ascontiguousarray(w2.reshape(NF, 128, D)).astype(bf)

    bq = ln1_b @ wq_p
    bk = ln1_b @ wk_p
    bv = ln1_b @ wv_flat
    bproj_eff = b_proj + bv @ w_proj
    b1_eff = ln2_b @ w1 + b1
    brows = np.concatenate([bq, bk, bproj_eff, b2]).reshape(1, 4 * D).astype(bf)
    b1t = np.ascontiguousarray(b1_eff.reshape(NF, 128).T).astype(f32)

    # rope tables: [128, NT, 4, HS]; 4 = (cos_q, sin_q, cos_k, sin_k)
    t = np.arange(T, dtype=f32)
    th = (1.0 / 10000.0 ** (np.arange(0, HS, 2, dtype=f32) / f32(HS))).astype(f32)
    ang = t[:, None] * th[None, :]
    cos = np.concatenate([np.cos(ang), np.cos(ang)], 1)           # [T, HS]
    sin = np.concatenate([-np.sin(ang), np.sin(ang)], 1)
    sc = f32(HS) ** f32(-0.5)
    rope = np.stack([cos * sc, sin * sc, cos, sin], 1)            # [T, 4, HS]
    rope_h = np.ascontiguousarray(
        rope.reshape(NT, 128, 4, HS).transpose(1, 0, 2, 3)).astype(bf)

    # causal 0/1 pair-masks: pair 0 = s-tiles (j=0, j=1), pair 1 = (j=2, j=3)
    sl = np.arange(128)[:, None]
    ql = np.arange(512)[None, :]
    m4 = [(j * 128 + sl <= ql).astype(bf) for j in range(4)]
    mask_h = np.stack([np.concatenate([m4[0], m4[1]], 1),
                       np.concatenate([m4[2], m4[3]], 1)])        # [2, 128, 1024]
    mask_h = np.ascontiguousarray(mask_h.transpose(1, 0, 2))      # [128, 2, 1024]

    common = {
        "wq": wq_h, "wk": wk_h, "wv": wv_h, "wp": wp_h,
        "w1": w1_h, "w2": w2_h,
        "rope": rope_h, "mask": mask_h,
        "ones64": np.ones((128, 64), bf),
        "onesrow": np.ones((1, 128), bf),
        "brows": brows, "b1t": b1t,
    }
    in_maps = [dict(common, x=np.ascontiguousarray(x[b])) for b in range(B)]
    return in_maps


_NC_CACHE = {}


def get_nc():
    if "nc" not in _NC_CACHE:
        _NC_CACHE["nc"] = build_kernel()
    return _NC_CACHE["nc"]


def kernel(**inputs):
    nc = get_nc()
    in_maps = _prep_inputs(inputs)
    res = run_bass_kernel_spmd(nc, in_maps, core_ids=list(range(NCORES)))
    out = np.stack([res.results[i]["out"] for i in range(NCORES)])
    return out.astype(np.float32)


# revision 7
# speedup vs baseline: 1.1987x; 1.0221x over previous
"""Trainium2 Bass kernel for a dense transformer block (pre-LN, causal, RoPE).

Sharding: data-parallel over batch. B=8 batch elements, 8 NeuronCores, one
batch element per core; weights replicated. No collectives needed.

Per-core single-batch block (T=1024, D=1024, H=16, HS=64, F=4096), bf16
matmul inputs with f32 PSUM accumulation:
  LN1 (f32 stats; rstd = exp(-0.5*ln(var+eps)) keeps ACT on one table set)
  -> zn bf16 -> DMA-transpose -> znT
  Q/K = znT @ wq/wk (hs-cols permuted even|odd, ln1_w folded) + RoPE (host
  cos/sin tables, HS**-0.5 q-scale folded) -> DMA-transpose -> QT/KT [ch, T]
  V = znT @ wv -> Vslab [T, ch]
  attention processed in head PAIRS (even head -> PE cols 0:63, odd head ->
  cols 64:127 via tile_position col-tiling): scoresT[s,q] pairs share one
  2-bank PSUM tile so exp runs once per [128,1024]; causal 0/1 pair-mask
  multiply; attn@V writes o_h|o_h' into one PSUM tile and a ones-matmul
  pair writes the softmax denominators into a second tile; normalize via
  exp(-ln(d)) on ACT + one DVE multiply per head-pair.
  proj + residual, LN2, FFN (relu+bias as one DVE tensor_scalar add/max),
  final residual, out f32.  All LN affine params and biases are folded
  host-side into the weight matrices / K=1 ones-row bias matmuls.
"""

import os
import sys
import numpy as np

for _p in ("/opt/trn_rl_repo", "/root/.axon_site/_ro/trn_rl_repo"):
    if os.path.isdir(_p) and _p not in sys.path:
        sys.path.append(_p)

import ml_dtypes

import concourse.bass as bass
import concourse.tile as tile
from concourse import bacc, mybir
from concourse.bass import ts
from concourse.bass_utils import run_bass_kernel_spmd

BF16 = mybir.dt.bfloat16
F32 = mybir.dt.float32
AF = mybir.ActivationFunctionType
ALU = mybir.AluOpType

B, T, D, H, HS, F = 8, 1024, 1024, 16, 64, 4096
NT = T // 128   # 8 T-tiles
ND = D // 128   # 8 D-chunks
NF = F // 128   # 32 F-chunks
NCORES = 8


def _bcast_heads(ap2d, nheads=H):
    """[128, J] AP -> [128, nheads, J] broadcast along a step-0 middle dim."""
    return bass.AP(ap2d.tensor, ap2d.offset, [ap2d.ap[0], [0, nheads], ap2d.ap[-1]])


def build_kernel():
    import contextlib

    nc = bacc.Bacc("TRN2", target_bir_lowering=False, debug=False,
                   num_devices=NCORES)

    # ---- external I/O ------------------------------------------------------
    xd = nc.dram_tensor("x", [T, D], F32, kind="ExternalInput")
    wq_d = nc.dram_tensor("wq", [128, ND, D], BF16, kind="ExternalInput")
    wk_d = nc.dram_tensor("wk", [128, ND, D], BF16, kind="ExternalInput")
    wv_d = nc.dram_tensor("wv", [128, ND, D], BF16, kind="ExternalInput")
    wp_d = nc.dram_tensor("wp", [128, ND, D], BF16, kind="ExternalInput")
    w1_d = nc.dram_tensor("w1", [ND, 128, F], BF16, kind="ExternalInput")
    w2_d = nc.dram_tensor("w2", [NF, 128, D], BF16, kind="ExternalInput")
    rope_d = nc.dram_tensor("rope", [128, NT, 4, HS], BF16, kind="ExternalInput")
    mask_d = nc.dram_tensor("mask", [128, 2, 1024], BF16, kind="ExternalInput")
    ones64_d = nc.dram_tensor("ones64", [128, 64], BF16, kind="ExternalInput")
    onesrow_d = nc.dram_tensor("onesrow", [1, 128], BF16, kind="ExternalInput")
    brows_d = nc.dram_tensor("brows", [1, 4 * D], BF16, kind="ExternalInput")
    b1t_d = nc.dram_tensor("b1t", [128, NF], F32, kind="ExternalInput")
    out_d = nc.dram_tensor("out", [T, D], F32, kind="ExternalOutput")

    xr = xd.ap().rearrange("(a p) d -> p a d", p=128)       # [128, NT, D]
    outr = out_d.ap().rearrange("(a p) d -> p a d", p=128)  # [128, NT, D]

    with tile.TileContext(nc) as tc:
        ctx = contextlib.ExitStack()
        with ctx:
            consts = ctx.enter_context(tc.tile_pool(name="consts", bufs=1))
            slabs = ctx.enter_context(tc.tile_pool(name="slabs", bufs=4))
            xpool = ctx.enter_context(tc.tile_pool(name="xpool", bufs=1))
            small = ctx.enter_context(tc.tile_pool(name="small", bufs=3))
            psA = ctx.enter_context(  # 2-bank tiles: QKV/proj/score-pairs
                tc.tile_pool(name="psA", bufs=3, space="PSUM"))
            psB = ctx.enter_context(  # 1-bank tiles: attnV, FFN
                tc.tile_pool(name="psB", bufs=2, space="PSUM"))

            # ---- global constants -----------------------------------------
            ones64 = consts.tile([128, 64], BF16)
            nc.sync.dma_start(out=ones64, in_=ones64_d.ap())
            onesrow = consts.tile([1, 128], BF16)
            nc.sync.dma_start(out=onesrow, in_=onesrow_d.ap())
            brows = consts.tile([1, 4 * D], BF16)  # bq | bk | bproj | b2
            nc.sync.dma_start(out=brows, in_=brows_d.ap())
            b1t = consts.tile([128, NF], F32)
            nc.sync.dma_start(out=b1t, in_=b1t_d.ap())
            eps = consts.tile([128, 1], F32)
            nc.vector.memset(eps, 1e-5)

            # ---- x in ------------------------------------------------------
            x_sb = xpool.tile([128, NT, D], F32)
            for xc in range(4):
                nc.sync.dma_start(out=x_sb[:, 2 * xc:2 * xc + 2, :],
                                  in_=xr[:, 2 * xc:2 * xc + 2, :])

            # ---- layernorm (standardize only; affine folded host-side) -----
            def layernorm_tile(src_sb, dst_bf16, tt):
                    xt = src_sb[:, tt, :]
                    stats = small.tile([128, 2, 6], F32, tag="lnstats")
                    for g in range(2):
                        nc.vector.bn_stats(out=stats[:, g, :],
                                           in_=xt[:, g * 512:(g + 1) * 512])
                    mv = small.tile([128, 2], F32, tag="lnmv")
                    nc.vector.bn_aggr(out=mv, in_=stats)
                    # rstd = exp(-0.5 * ln(var + eps)); ln+exp share one ACT
                    # table set with the attention exp -> no table thrash.
                    std = small.tile([128, 2], F32, tag="lnstd")
                    nc.scalar.activation(out=std[:, 1:2], in_=mv[:, 1:2],
                                         func=AF.Ln, bias=eps, scale=1.0)
                    nc.scalar.activation(out=std[:, 0:1], in_=std[:, 1:2],
                                         func=AF.Exp, scale=-0.5)
                    nc.vector.tensor_scalar(
                        out=dst_bf16[:, tt, :], in0=xt,
                        scalar1=mv[:, 0:1], scalar2=std[:, 0:1],
                        op0=ALU.subtract, op1=ALU.mult)

            znT = slabs.tile([128, ND, T], BF16, tag="slab")
            zn = slabs.tile([128, NT, D], BF16, tag="slab")
            for tt in range(NT):
                layernorm_tile(x_sb, zn, tt)
                nc.sync.dma_start(out=znT[:, :, ts(tt, 128)],
                                  in_=zn[:, tt, :], transpose=True)

            QT = slabs.tile([128, ND, T], BF16, tag="slab")
            KT = slabs.tile([128, ND, T], BF16, tag="slab")

            # ============ attention super-phase (scoped pool) ==============
            actx = contextlib.ExitStack()
            with actx:
                apool = actx.enter_context(tc.tile_pool(name="apool", bufs=2))
                ppool = actx.enter_context(tc.tile_pool(name="ppool", bufs=9))

                rope_sb = apool.tile([128, NT, 4, HS], BF16, tag="rope")
                nc.sync.dma_start(out=rope_sb, in_=rope_d.ap())
                mask_sb = apool.tile([128, 2, 1024], BF16, tag="mask")
                nc.sync.dma_start(out=mask_sb, in_=mask_d.ap())

                def qkv_proj(w_dram, brow_idx):
                    w_sb = apool.tile([128, ND, D], BF16, tag="w")
                    for wc in range(4):
                        nc.sync.dma_start(
                            out=w_sb[:, 2 * wc:2 * wc + 2, :],
                            in_=w_dram.ap()[:, 2 * wc:2 * wc + 2, :])
                    for tt in range(NT):
                        ps = psA.tile([128, 1024], F32, tag="A")
                        last = ND - 1
                        for c in range(ND):
                            fin = (c == last and brow_idx is None)
                            lhsT = znT[:, c, ts(tt, 128)]
                            nc.tensor.matmul(ps[:, 0:512], lhsT,
                                             w_sb[:, c, 0:512],
                                             start=(c == 0), stop=fin)
                            nc.tensor.matmul(ps[:, 512:1024], lhsT,
                                             w_sb[:, c, 512:1024],
                                             start=(c == 0), stop=fin)
                        if brow_idx is not None:
                            o = brow_idx * D
                            nc.tensor.matmul(ps[:, 0:512], onesrow,
                                             brows[0:1, o:o + 512],
                                             start=False, stop=True)
                            nc.tensor.matmul(ps[:, 512:1024], onesrow,
                                             brows[0:1, o + 512:o + 1024],
                                             start=False, stop=True)
                        yield tt, ps

                # -- Q then K: copy out of PSUM, rope, DMA-transpose
                for w_dram, brow_idx, dstT, tblc, tbls in (
                        (wq_d, 0, QT, 0, 1), (wk_d, 1, KT, 2, 3)):
                    for tt, ps in qkv_proj(w_dram, brow_idx):
                        raw = apool.tile([128, D], BF16, tag="qkraw")
                        nc.scalar.activation(out=raw, in_=ps, func=AF.Copy)
                        rot = apool.tile([128, D], BF16, tag="qkrot")
                        rv = rot.rearrange("p (h x j) -> p h x j", h=H, x=2)
                        qv = raw.rearrange("p (h x j) -> p h x j", h=H, x=2)
                        cos_t = _bcast_heads(rope_sb[:, tt, tblc, :])
                        cos_t = bass.AP(cos_t.tensor, cos_t.offset,
                                        cos_t.ap[:2] + [[32, 2], [1, 32]])
                        sin_e = _bcast_heads(rope_sb[:, tt, tbls, 0:32])
                        sin_o = _bcast_heads(rope_sb[:, tt, tbls, 32:64])
                        tmp = apool.tile([128, D], BF16, tag="qktmp")
                        tv = tmp.rearrange("p (h x j) -> p h x j", h=H, x=2)
                        # tmp = swap_halves(q) * (+-sin)
                        nc.vector.tensor_mul(out=tv[:, :, 0, :],
                                             in0=qv[:, :, 1, :], in1=sin_e)
                        nc.vector.tensor_mul(out=tv[:, :, 1, :],
                                             in0=qv[:, :, 0, :], in1=sin_o)
                        nc.vector.tensor_mul(out=rv, in0=qv, in1=cos_t)
                        nc.vector.tensor_add(out=rot, in0=rot, in1=tmp)
                        nc.sync.dma_start(out=dstT[:, :, ts(tt, 128)],
                                          in_=rot, transpose=True)

                # -- V (plain copy; ln1_b contribution folded into b_proj)
                Vs = slabs.tile([128, NT, D], BF16, tag="slab")
                for tt, ps in qkv_proj(wv_d, None):
                    nc.scalar.activation(out=Vs[:, tt, :], in_=ps, func=AF.Copy)

                # -- attention: 2 head-pairs (4 heads) per group; the two
                # pairs share one 2-bank denominator tile so ln/exp run once.
                oT = slabs.tile([128, ND, T], BF16, tag="slab")
                for qb in range(2):
                    n_sc = 4 * (qb + 1)
                    qsl = slice(qb * 512, (qb + 1) * 512)
                    for cg in range(ND // 2):       # cidx pair (2cg, 2cg+1)
                        dp2 = psA.tile([128, 1024], F32, tag="A")
                        pos = []
                        for ci in range(2):
                            cidx = 2 * cg + ci
                            h0 = 2 * cidx
                            kT0 = KT[0:64, cidx, :]
                            kT1 = KT[64:128, cidx, :]
                            qT0 = QT[0:64, cidx, qsl]
                            qT1 = QT[64:128, cidx, qsl]
                            pt0, pt1 = [], []
                            for spr in range(n_sc // 2):
                                sc0, sc1 = 2 * spr, 2 * spr + 1
                                for kT_h, qT_h, plist in ((kT0, qT0, pt0),
                                                          (kT1, qT1, pt1)):
                                    ps = psA.tile([128, 1024], F32, tag="A")
                                    nc.tensor.matmul(ps[:, 0:512],
                                                     kT_h[:, ts(sc0, 128)], qT_h,
                                                     start=True, stop=True)
                                    nc.tensor.matmul(ps[:, 512:1024],
                                                     kT_h[:, ts(sc1, 128)], qT_h,
                                                     start=True, stop=True)
                                    P = ppool.tile([128, 1024], BF16, tag="P")
                                    nc.scalar.activation(out=P, in_=ps,
                                                         func=AF.Exp)
                                    if spr >= 2 * qb:   # diagonal-crossing
                                        nc.vector.tensor_mul(
                                            out=P, in0=P,
                                            in1=mask_sb[:, spr - 2 * qb, :])
                                    plist.append(P)
                            po = psB.tile([128, 512], F32, tag="B")
                            pos.append(po)
                            dsl = slice(ci * 512, ci * 512 + 512)
                            for sc in range(n_sc):
                                st, sp = (sc == 0), (sc == n_sc - 1)
                                o0 = (sc % 2) * 512
                                P0 = pt0[sc // 2][:, o0:o0 + 512]
                                P1 = pt1[sc // 2][:, o0:o0 + 512]
                                nc.tensor.matmul(po[0:64, :],
                                                 Vs[:, sc, h0 * 64:h0 * 64 + 64],
                                                 P0, start=st, stop=sp,
                                                 tile_position=(0, 0))
                                nc.tensor.matmul(
                                    po[64:128, :],
                                    Vs[:, sc, h0 * 64 + 64:h0 * 64 + 128],
                                    P1, start=st, stop=sp,
                                    tile_position=(0, 64))
                                nc.tensor.matmul(dp2[0:64, dsl], ones64, P0,
                                                 start=st, stop=sp,
                                                 tile_position=(0, 0))
                                nc.tensor.matmul(dp2[64:128, dsl], ones64, P1,
                                                 start=st, stop=sp,
                                                 tile_position=(0, 64))
                        lnd = apool.tile([128, 1024], F32, tag="lnd")
                        nc.scalar.activation(out=lnd, in_=dp2, func=AF.Ln)
                        rec = apool.tile([128, 1024], BF16, tag="rec")
                        nc.scalar.activation(out=rec, in_=lnd, func=AF.Exp,
                                             scale=-1.0)
                        for ci in range(2):
                            cidx = 2 * cg + ci
                            nc.vector.tensor_mul(
                                out=oT[:, cidx, qsl], in0=pos[ci],
                                in1=rec[:, ci * 512:ci * 512 + 512])

                # -- proj + residual -> x2 (bf16)
                wp_sb = apool.tile([128, ND, D], BF16, tag="w")
                for wc in range(4):
                    nc.sync.dma_start(out=wp_sb[:, 2 * wc:2 * wc + 2, :],
                                      in_=wp_d.ap()[:, 2 * wc:2 * wc + 2, :])
                x2 = slabs.tile([128, NT, D], BF16, tag="slab")
                z2 = slabs.tile([128, NT, D], BF16, tag="slab")
                z2T = slabs.tile([128, ND, T], BF16, tag="slab")
                for tt in range(NT):
                    ps = psA.tile([128, 1024], F32, tag="A")
                    for c in range(ND):
                        lhsT = oT[:, c, ts(tt, 128)]
                        nc.tensor.matmul(ps[:, 0:512], lhsT,
                                         wp_sb[:, c, 0:512],
                                         start=(c == 0), stop=False)
                        nc.tensor.matmul(ps[:, 512:1024], lhsT,
                                         wp_sb[:, c, 512:1024],
                                         start=(c == 0), stop=False)
                    nc.tensor.matmul(ps[:, 0:512], onesrow,
                                     brows[0:1, 2 * D:2 * D + 512],
                                     start=False, stop=True)
                    nc.tensor.matmul(ps[:, 512:1024], onesrow,
                                     brows[0:1, 2 * D + 512:3 * D],
                                     start=False, stop=True)
                    nc.vector.tensor_add(out=x2[:, tt, :], in0=ps,
                                         in1=x_sb[:, tt, :])
                    # LN2 + transpose interleaved per tile so FFN1 can start
                    # as soon as the first T-half is standardized.
                    layernorm_tile(x2, z2, tt)
                    nc.sync.dma_start(out=z2T[:, :, ts(tt, 128)],
                                      in_=z2[:, tt, :], transpose=True)

            # ============ FFN super-phase (scoped pool) ====================
            fctx = contextlib.ExitStack()
            with fctx:
                fpool = fctx.enter_context(tc.tile_pool(name="fpool", bufs=1))
                w1pool = fctx.enter_context(tc.tile_pool(name="w1pool", bufs=2))
                w2pool = fctx.enter_context(tc.tile_pool(name="w2pool", bufs=5))
                opool = fctx.enter_context(tc.tile_pool(name="opool", bufs=4))
                for tb in range(2):
                    tbs = slice(tb * 512, (tb + 1) * 512)
                    # FFN1 half: hT[f, t-half] = relu(w1.T @ z2T + b1) on DVE
                    hTh = fpool.tile([128, NF, 512], BF16, tag="hTh")
                    for mg in range(NF // 4):
                        w1g = w1pool.tile([128, ND, 512], BF16, tag="w1g")
                        nc.sync.dma_start(
                            out=w1g,
                            in_=w1_d.ap()[:, :, mg * 512:(mg + 1) * 512]
                            .rearrange("c p f -> p c f"))
                        for mi in range(4):
                            m = mg * 4 + mi
                            ps = psB.tile([128, 512], F32, tag="B")
                            for c in range(ND):
                                nc.tensor.matmul(
                                    ps, w1g[:, c, ts(mi, 128)],
                                    z2T[:, c, tbs],
                                    start=(c == 0), stop=(c == ND - 1))
                            nc.vector.tensor_scalar(
                                out=hTh[:, m, :], in0=ps,
                                scalar1=b1t[:, m:m + 1], scalar2=0.0,
                                op0=ALU.add, op1=ALU.max)
                    # FFN2 half + residual -> out
                    for db in range(2):
                        w2cs = []
                        for cg in range(4):
                            w2c = w2pool.tile([128, 8, 512], BF16, tag="w2c")
                            nc.sync.dma_start(
                                out=w2c,
                                in_=w2_d.ap()[cg * 8:(cg + 1) * 8, :,
                                              db * 512:(db + 1) * 512]
                                .rearrange("c p f -> p c f"))
                            w2cs.append(w2c)
                        for tl in range(4):
                            tt = tb * 4 + tl
                            ps = psB.tile([128, 512], F32, tag="B")
                            for c in range(NF):
                                nc.tensor.matmul(ps, hTh[:, c, ts(tl, 128)],
                                                 w2cs[c // 8][:, c % 8, :],
                                                 start=(c == 0), stop=False)
                            o = 3 * D + db * 512
                            nc.tensor.matmul(ps, onesrow, brows[0:1, o:o + 512],
                                             start=False, stop=True)
                            ot = opool.tile([128, 512], F32, tag="ot")
                            nc.vector.tensor_add(
                                out=ot, in0=ps,
                                in1=x2[:, tt, db * 512:(db + 1) * 512])
                            nc.sync.dma_start(
                                out=outr[:, tt, db * 512:(db + 1) * 512],
                                in_=ot)

    nc.compile()
    return nc


def _prep_inputs(inputs):
    """Host-side preprocessing: fold LN affine, permute rope cols, cast bf16."""
    f32 = np.float32
    x = np.asarray(inputs["x"], f32)
    wq = np.asarray(inputs["wq"], f32)
    wk = np.asarray(inputs["wk"], f32)
    wv = np.asarray(inputs["wv"], f32)
    w_proj = np.asarray(inputs["w_proj"], f32)
    b_proj = np.asarray(inputs["b_proj"], f32)
    ln1_w = np.asarray(inputs["ln1_w"], f32)
    ln1_b = np.asarray(inputs["ln1_b"], f32)
    ln2_w = np.asarray(inputs["ln2_w"], f32)
    ln2_b = np.asarray(inputs["ln2_b"], f32)
    w1 = np.asarray(inputs["w1"], f32)
    b1 = np.asarray(inputs["b1"], f32)
    w2 = np.asarray(inputs["w2"], f32)
    b2 = np.asarray(inputs["b2"], f32)

    bf = ml_dtypes.bfloat16
    perm = np.concatenate([np.arange(0, HS, 2), np.arange(1, HS, 2)])
    idx = (np.arange(H)[:, None] * HS + perm[None, :]).reshape(-1)

    wq_flat = wq.transpose(1, 0, 2).reshape(D, H * HS)
    wk_flat = wk.transpose(1, 0, 2).reshape(D, H * HS)
    wv_flat = wv.transpose(1, 0, 2).reshape(D, H * HS)
    wq_p = wq_flat[:, idx]
    wk_p = wk_flat[:, idx]

    def wlayout(w):  # [D, D] -> [128, ND, D]  (p = d_in, c = d_chunk)
        return np.ascontiguousarray(
            w.reshape(ND, 128, D).transpose(1, 0, 2)).astype(bf)

    wq_h = wlayout(ln1_w[:, None] * wq_p)
    wk_h = wlayout(ln1_w[:, None] * wk_p)
    wv_h = wlayout(ln1_w[:, None] * wv_flat)
    wp_h = wlayout(w_proj)
    w1_h = np.ascontiguousarray(
        (ln2_w[:, None] * w1).reshape(ND, 128, F)).astype(bf)
    w2_h = np.ascontiguousarray(w2.reshape(NF, 128, D)).astype(bf)

    bq = ln1_b @ wq_p
    bk = ln1_b @ wk_p
    bv = ln1_b @ wv_flat
    bproj_eff = b_proj + bv @ w_proj
    b1_eff = ln2_b @ w1 + b1
    brows = np.concatenate([bq, bk, bproj_eff, b2]).reshape(1, 4 * D).astype(bf)
    b1t = np.ascontiguousarray(b1_eff.reshape(NF, 128).T).astype(f32)

    # rope tables: [128, NT, 4, HS]; 4 = (cos_q, sin_q, cos_k, sin_k)
    t = np.arange(T, dtype=f32)
    th = (1.0 / 10000.0 ** (np.arange(0, HS, 2, dtype=f32) / f32(HS))).astype(f32)
    ang = t[:, None] * th[None, :]
    cos = np.concatenate([np.cos(ang), np.cos(ang)], 1)           # [T, HS]
    sin = np.concatenate([-np.sin(ang), np.sin(ang)], 1)
    sc = f32(HS) ** f32(-0.5)
    rope = np.stack([cos * sc, sin * sc, cos, sin], 1)            # [T, 4, HS]
    rope_h = np.ascontiguousarray(
        rope.reshape(NT, 128, 4, HS).transpose(1, 0, 2, 3)).astype(bf)

    # causal 0/1 pair-masks: pair 0 = s-tiles (j=0, j=1), pair 1 = (j=2, j=3)
    sl = np.arange(128)[:, None]
    ql = np.arange(512)[None, :]
    m4 = [(j * 128 + sl <= ql).astype(bf) for j in range(4)]
    mask_h = np.stack([np.concatenate([m4[0], m4[1]], 1),
                       np.concatenate([m4[2], m4[3]], 1)])        # [2, 128, 1024]
    mask_h = np.ascontiguousarray(mask_h.transpose(1, 0, 2))      # [128, 2, 1024]

    common = {
        "wq": wq_h, "wk": wk_h, "wv": wv_h, "wp": wp_h,
        "w1": w1_h, "w2": w2_h,
        "rope": rope_h, "mask": mask_h,
        "ones64": np.ones((128, 64), bf),
        "onesrow": np.ones((1, 128), bf),
        "brows": brows, "b1t": b1t,
    }
    in_maps = [dict(common, x=np.ascontiguousarray(x[b])) for b in range(B)]
    return in_maps


_NC_CACHE = {}


def get_nc():
    if "nc" not in _NC_CACHE:
        _NC_CACHE["nc"] = build_kernel()
    return _NC_CACHE["nc"]


def kernel(**inputs):
    nc = get_nc()
    in_maps = _prep_inputs(inputs)
    res = run_bass_kernel_spmd(nc, in_maps, core_ids=list(range(NCORES)))
    out = np.stack([res.results[i]["out"] for i in range(NCORES)])
    return out.astype(np.float32)


# revision 8
# speedup vs baseline: 1.2186x; 1.0166x over previous
"""Trainium2 Bass kernel for a dense transformer block (pre-LN, causal, RoPE).

Sharding: data-parallel over batch. B=8 batch elements, 8 NeuronCores, one
batch element per core; weights replicated. No collectives needed.

Per-core single-batch block (T=1024, D=1024, H=16, HS=64, F=4096), bf16
matmul inputs with f32 PSUM accumulation:
  LN1 (f32 stats; rstd = exp(-0.5*ln(var+eps)) keeps ACT on one table set)
  -> zn bf16 -> DMA-transpose -> znT
  Q/K = znT @ wq/wk (hs-cols permuted even|odd, ln1_w folded) + RoPE (host
  cos/sin tables, HS**-0.5 q-scale folded) -> DMA-transpose -> QT/KT [ch, T]
  V = znT @ wv -> Vslab [T, ch]
  attention processed in head PAIRS (even head -> PE cols 0:63, odd head ->
  cols 64:127 via tile_position col-tiling): scoresT[s,q] pairs share one
  2-bank PSUM tile so exp runs once per [128,1024]; causal 0/1 pair-mask
  multiply; attn@V writes o_h|o_h' into one PSUM tile and a ones-matmul
  pair writes the softmax denominators into a second tile; normalize via
  exp(-ln(d)) on ACT + one DVE multiply per head-pair.
  proj + residual, LN2, FFN (relu+bias as one DVE tensor_scalar add/max),
  final residual, out f32.  All LN affine params and biases are folded
  host-side into the weight matrices / K=1 ones-row bias matmuls.
"""

import os
import sys
import numpy as np

for _p in ("/opt/trn_rl_repo", "/root/.axon_site/_ro/trn_rl_repo"):
    if os.path.isdir(_p) and _p not in sys.path:
        sys.path.append(_p)

import ml_dtypes

import concourse.bass as bass
import concourse.tile as tile
from concourse import bacc, mybir
from concourse.bass import ts
from concourse.bass_utils import run_bass_kernel_spmd

BF16 = mybir.dt.bfloat16
F32 = mybir.dt.float32
AF = mybir.ActivationFunctionType
ALU = mybir.AluOpType

B, T, D, H, HS, F = 8, 1024, 1024, 16, 64, 4096
NT = T // 128   # 8 T-tiles
ND = D // 128   # 8 D-chunks
NF = F // 128   # 32 F-chunks
NCORES = 8


def _bcast_heads(ap2d, nheads=H):
    """[128, J] AP -> [128, nheads, J] broadcast along a step-0 middle dim."""
    return bass.AP(ap2d.tensor, ap2d.offset, [ap2d.ap[0], [0, nheads], ap2d.ap[-1]])


def _patch_act_tables():
    """Force Exp and Ln onto the combined natural_log_exp_and_others set so
    the whole kernel runs on ONE resident ACT table (no 1.3us reloads)."""
    from concourse import hw_specs, bacc as _bacc
    orig = hw_specs.get_activation_tables
    if getattr(hw_specs, "_act_tables_patched", False):
        return
    def patched(arch):
        t = orig(arch)
        if "natural_log_exp_and_others" in t:
            for name in ("exp_and_others", "natural_log", "exp_and_friends"):
                if name in t:
                    t[name] = set()
        return t
    hw_specs.get_activation_tables = patched
    _bacc.get_activation_tables = patched
    hw_specs._act_tables_patched = True


def build_kernel():
    import contextlib

    _patch_act_tables()

    nc = bacc.Bacc("TRN2", target_bir_lowering=False, debug=False,
                   num_devices=NCORES)

    # ---- external I/O ------------------------------------------------------
    xd = nc.dram_tensor("x", [T, D], F32, kind="ExternalInput")
    wq_d = nc.dram_tensor("wq", [128, ND, D], BF16, kind="ExternalInput")
    wk_d = nc.dram_tensor("wk", [128, ND, D], BF16, kind="ExternalInput")
    wv_d = nc.dram_tensor("wv", [128, ND, D], BF16, kind="ExternalInput")
    wp_d = nc.dram_tensor("wp", [128, ND, D], BF16, kind="ExternalInput")
    w1_d = nc.dram_tensor("w1", [ND, 128, F], BF16, kind="ExternalInput")
    w2_d = nc.dram_tensor("w2", [NF, 128, D], BF16, kind="ExternalInput")
    rope_d = nc.dram_tensor("rope", [128, NT, 4, HS], BF16, kind="ExternalInput")
    mask_d = nc.dram_tensor("mask", [128, 2, 1024], BF16, kind="ExternalInput")
    ones64_d = nc.dram_tensor("ones64", [128, 64], BF16, kind="ExternalInput")
    onesrow_d = nc.dram_tensor("onesrow", [1, 128], BF16, kind="ExternalInput")
    brows_d = nc.dram_tensor("brows", [1, 4 * D], BF16, kind="ExternalInput")
    b1t_d = nc.dram_tensor("b1t", [128, NF], F32, kind="ExternalInput")
    out_d = nc.dram_tensor("out", [T, D], F32, kind="ExternalOutput")

    xr = xd.ap().rearrange("(a p) d -> p a d", p=128)       # [128, NT, D]
    outr = out_d.ap().rearrange("(a p) d -> p a d", p=128)  # [128, NT, D]

    with tile.TileContext(nc) as tc:
        ctx = contextlib.ExitStack()
        with ctx:
            consts = ctx.enter_context(tc.tile_pool(name="consts", bufs=1))
            slabs = ctx.enter_context(tc.tile_pool(name="slabs", bufs=4))
            xpool = ctx.enter_context(tc.tile_pool(name="xpool", bufs=1))
            small = ctx.enter_context(tc.tile_pool(name="small", bufs=3))
            psA = ctx.enter_context(  # 2-bank tiles: QKV/proj/score-pairs
                tc.tile_pool(name="psA", bufs=3, space="PSUM"))
            psB = ctx.enter_context(  # 1-bank tiles: attnV, FFN
                tc.tile_pool(name="psB", bufs=2, space="PSUM"))

            # ---- global constants -----------------------------------------
            ones64 = consts.tile([128, 64], BF16)
            nc.sync.dma_start(out=ones64, in_=ones64_d.ap())
            onesrow = consts.tile([1, 128], BF16)
            nc.sync.dma_start(out=onesrow, in_=onesrow_d.ap())
            brows = consts.tile([1, 4 * D], BF16)  # bq | bk | bproj | b2
            nc.sync.dma_start(out=brows, in_=brows_d.ap())
            b1t = consts.tile([128, NF], F32)
            nc.sync.dma_start(out=b1t, in_=b1t_d.ap())
            eps = consts.tile([128, 1], F32)
            nc.vector.memset(eps, 1e-5)

            # ---- x in ------------------------------------------------------
            x_sb = xpool.tile([128, NT, D], F32)
            for xc in range(4):
                nc.sync.dma_start(out=x_sb[:, 2 * xc:2 * xc + 2, :],
                                  in_=xr[:, 2 * xc:2 * xc + 2, :])

            # ---- layernorm (standardize only; affine folded host-side) -----
            def layernorm_tile(src_sb, dst_bf16, tt):
                    xt = src_sb[:, tt, :]
                    stats = small.tile([128, 2, 6], F32, tag="lnstats")
                    for g in range(2):
                        nc.vector.bn_stats(out=stats[:, g, :],
                                           in_=xt[:, g * 512:(g + 1) * 512])
                    mv = small.tile([128, 2], F32, tag="lnmv")
                    nc.vector.bn_aggr(out=mv, in_=stats)
                    # rstd = exp(-0.5 * ln(var + eps)); ln+exp share one ACT
                    # table set with the attention exp -> no table thrash.
                    std = small.tile([128, 2], F32, tag="lnstd")
                    nc.scalar.activation(out=std[:, 1:2], in_=mv[:, 1:2],
                                         func=AF.Ln, bias=eps, scale=1.0)
                    nc.scalar.activation(out=std[:, 0:1], in_=std[:, 1:2],
                                         func=AF.Exp, scale=-0.5)
                    nc.vector.tensor_scalar(
                        out=dst_bf16[:, tt, :], in0=xt,
                        scalar1=mv[:, 0:1], scalar2=std[:, 0:1],
                        op0=ALU.subtract, op1=ALU.mult)

            znT = slabs.tile([128, ND, T], BF16, tag="slab")
            zn = slabs.tile([128, NT, D], BF16, tag="slab")
            for tt in range(NT):
                layernorm_tile(x_sb, zn, tt)
                nc.scalar.dma_start(out=znT[:, :, ts(tt, 128)],
                                  in_=zn[:, tt, :], transpose=True)

            QT = slabs.tile([128, ND, T], BF16, tag="slab")
            KT = slabs.tile([128, ND, T], BF16, tag="slab")

            # ============ attention super-phase (scoped pool) ==============
            actx = contextlib.ExitStack()
            with actx:
                apool = actx.enter_context(tc.tile_pool(name="apool", bufs=2))
                ppool = actx.enter_context(tc.tile_pool(name="ppool", bufs=9))

                rope_sb = apool.tile([128, NT, 4, HS], BF16, tag="rope")
                nc.sync.dma_start(out=rope_sb, in_=rope_d.ap())
                mask_sb = apool.tile([128, 2, 1024], BF16, tag="mask")
                nc.sync.dma_start(out=mask_sb, in_=mask_d.ap())

                def qkv_proj(w_dram, brow_idx):
                    w_sb = apool.tile([128, ND, D], BF16, tag="w")
                    for wc in range(4):
                        nc.sync.dma_start(
                            out=w_sb[:, 2 * wc:2 * wc + 2, :],
                            in_=w_dram.ap()[:, 2 * wc:2 * wc + 2, :])
                    for tt in range(NT):
                        ps = psA.tile([128, 1024], F32, tag="A")
                        last = ND - 1
                        for c in range(ND):
                            fin = (c == last and brow_idx is None)
                            lhsT = znT[:, c, ts(tt, 128)]
                            nc.tensor.matmul(ps[:, 0:512], lhsT,
                                             w_sb[:, c, 0:512],
                                             start=(c == 0), stop=fin)
                            nc.tensor.matmul(ps[:, 512:1024], lhsT,
                                             w_sb[:, c, 512:1024],
                                             start=(c == 0), stop=fin)
                        if brow_idx is not None:
                            o = brow_idx * D
                            nc.tensor.matmul(ps[:, 0:512], onesrow,
                                             brows[0:1, o:o + 512],
                                             start=False, stop=True)
                            nc.tensor.matmul(ps[:, 512:1024], onesrow,
                                             brows[0:1, o + 512:o + 1024],
                                             start=False, stop=True)
                        yield tt, ps

                # -- Q then K: copy out of PSUM, rope, DMA-transpose
                for w_dram, brow_idx, dstT, tblc, tbls in (
                        (wq_d, 0, QT, 0, 1), (wk_d, 1, KT, 2, 3)):
                    for tt, ps in qkv_proj(w_dram, brow_idx):
                        raw = apool.tile([128, D], BF16, tag="qkraw")
                        nc.scalar.activation(out=raw, in_=ps, func=AF.Copy)
                        rot = apool.tile([128, D], BF16, tag="qkrot")
                        rv = rot.rearrange("p (h x j) -> p h x j", h=H, x=2)
                        qv = raw.rearrange("p (h x j) -> p h x j", h=H, x=2)
                        cos_t = _bcast_heads(rope_sb[:, tt, tblc, :])
                        cos_t = bass.AP(cos_t.tensor, cos_t.offset,
                                        cos_t.ap[:2] + [[32, 2], [1, 32]])
                        sin_e = _bcast_heads(rope_sb[:, tt, tbls, 0:32])
                        sin_o = _bcast_heads(rope_sb[:, tt, tbls, 32:64])
                        tmp = apool.tile([128, D], BF16, tag="qktmp")
                        tv = tmp.rearrange("p (h x j) -> p h x j", h=H, x=2)
                        # tmp = swap_halves(q) * (+-sin)
                        nc.vector.tensor_mul(out=tv[:, :, 0, :],
                                             in0=qv[:, :, 1, :], in1=sin_e)
                        nc.vector.tensor_mul(out=tv[:, :, 1, :],
                                             in0=qv[:, :, 0, :], in1=sin_o)
                        nc.vector.tensor_mul(out=rv, in0=qv, in1=cos_t)
                        nc.vector.tensor_add(out=rot, in0=rot, in1=tmp)
                        nc.scalar.dma_start(out=dstT[:, :, ts(tt, 128)],
                                          in_=rot, transpose=True)

                # -- V (plain copy; ln1_b contribution folded into b_proj)
                Vs = slabs.tile([128, NT, D], BF16, tag="slab")
                for tt, ps in qkv_proj(wv_d, None):
                    nc.scalar.activation(out=Vs[:, tt, :], in_=ps, func=AF.Copy)

                # -- attention: 2 head-pairs (4 heads) per group; the two
                # pairs share one 2-bank denominator tile so ln/exp run once.
                oT = slabs.tile([128, ND, T], BF16, tag="slab")
                for qb in range(2):
                    n_sc = 4 * (qb + 1)
                    qsl = slice(qb * 512, (qb + 1) * 512)
                    for cg in range(ND // 2):       # cidx pair (2cg, 2cg+1)
                        dp2 = psA.tile([128, 1024], F32, tag="A")
                        pos = []
                        for ci in range(2):
                            cidx = 2 * cg + ci
                            h0 = 2 * cidx
                            kT0 = KT[0:64, cidx, :]
                            kT1 = KT[64:128, cidx, :]
                            qT0 = QT[0:64, cidx, qsl]
                            qT1 = QT[64:128, cidx, qsl]
                            pt0, pt1 = [], []
                            for spr in range(n_sc // 2):
                                sc0, sc1 = 2 * spr, 2 * spr + 1
                                for kT_h, qT_h, plist in ((kT0, qT0, pt0),
                                                          (kT1, qT1, pt1)):
                                    ps = psA.tile([128, 1024], F32, tag="A")
                                    nc.tensor.matmul(ps[:, 0:512],
                                                     kT_h[:, ts(sc0, 128)], qT_h,
                                                     start=True, stop=True)
                                    nc.tensor.matmul(ps[:, 512:1024],
                                                     kT_h[:, ts(sc1, 128)], qT_h,
                                                     start=True, stop=True)
                                    P = ppool.tile([128, 1024], BF16, tag="P")
                                    nc.scalar.activation(out=P, in_=ps,
                                                         func=AF.Exp)
                                    if spr >= 2 * qb:   # diagonal-crossing
                                        nc.vector.tensor_mul(
                                            out=P, in0=P,
                                            in1=mask_sb[:, spr - 2 * qb, :])
                                    plist.append(P)
                            po = psB.tile([128, 512], F32, tag="B")
                            pos.append(po)
                            dsl = slice(ci * 512, ci * 512 + 512)
                            for sc in range(n_sc):
                                st, sp = (sc == 0), (sc == n_sc - 1)
                                o0 = (sc % 2) * 512
                                P0 = pt0[sc // 2][:, o0:o0 + 512]
                                P1 = pt1[sc // 2][:, o0:o0 + 512]
                                nc.tensor.matmul(po[0:64, :],
                                                 Vs[:, sc, h0 * 64:h0 * 64 + 64],
                                                 P0, start=st, stop=sp,
                                                 tile_position=(0, 0))
                                nc.tensor.matmul(
                                    po[64:128, :],
                                    Vs[:, sc, h0 * 64 + 64:h0 * 64 + 128],
                                    P1, start=st, stop=sp,
                                    tile_position=(0, 64))
                                nc.tensor.matmul(dp2[0:64, dsl], ones64, P0,
                                                 start=st, stop=sp,
                                                 tile_position=(0, 0))
                                nc.tensor.matmul(dp2[64:128, dsl], ones64, P1,
                                                 start=st, stop=sp,
                                                 tile_position=(0, 64))
                        lnd = apool.tile([128, 1024], F32, tag="lnd")
                        nc.scalar.activation(out=lnd, in_=dp2, func=AF.Ln)
                        rec = apool.tile([128, 1024], BF16, tag="rec")
                        nc.scalar.activation(out=rec, in_=lnd, func=AF.Exp,
                                             scale=-1.0)
                        for ci in range(2):
                            cidx = 2 * cg + ci
                            nc.vector.tensor_mul(
                                out=oT[:, cidx, qsl], in0=pos[ci],
                                in1=rec[:, ci * 512:ci * 512 + 512])

                # -- proj + residual -> x2 (bf16)
                wp_sb = apool.tile([128, ND, D], BF16, tag="w")
                for wc in range(4):
                    nc.sync.dma_start(out=wp_sb[:, 2 * wc:2 * wc + 2, :],
                                      in_=wp_d.ap()[:, 2 * wc:2 * wc + 2, :])
                x2 = slabs.tile([128, NT, D], BF16, tag="slab")
                z2 = slabs.tile([128, NT, D], BF16, tag="slab")
                z2T = slabs.tile([128, ND, T], BF16, tag="slab")
                for tt in range(NT):
                    ps = psA.tile([128, 1024], F32, tag="A")
                    for c in range(ND):
                        lhsT = oT[:, c, ts(tt, 128)]
                        nc.tensor.matmul(ps[:, 0:512], lhsT,
                                         wp_sb[:, c, 0:512],
                                         start=(c == 0), stop=False)
                        nc.tensor.matmul(ps[:, 512:1024], lhsT,
                                         wp_sb[:, c, 512:1024],
                                         start=(c == 0), stop=False)
                    nc.tensor.matmul(ps[:, 0:512], onesrow,
                                     brows[0:1, 2 * D:2 * D + 512],
                                     start=False, stop=True)
                    nc.tensor.matmul(ps[:, 512:1024], onesrow,
                                     brows[0:1, 2 * D + 512:3 * D],
                                     start=False, stop=True)
                    nc.vector.tensor_add(out=x2[:, tt, :], in0=ps,
                                         in1=x_sb[:, tt, :])
                    # LN2 + transpose interleaved per tile so FFN1 can start
                    # as soon as the first T-half is standardized.
                    layernorm_tile(x2, z2, tt)
                    nc.scalar.dma_start(out=z2T[:, :, ts(tt, 128)],
                                      in_=z2[:, tt, :], transpose=True)

            # ============ FFN super-phase (scoped pool) ====================
            fctx = contextlib.ExitStack()
            with fctx:
                fpool = fctx.enter_context(tc.tile_pool(name="fpool", bufs=1))
                w1pool = fctx.enter_context(tc.tile_pool(name="w1pool", bufs=2))
                w2pool = fctx.enter_context(tc.tile_pool(name="w2pool", bufs=5))
                opool = fctx.enter_context(tc.tile_pool(name="opool", bufs=4))
                for tb in range(2):
                    tbs = slice(tb * 512, (tb + 1) * 512)
                    # FFN1 half: hT[f, t-half] = relu(w1.T @ z2T + b1) on DVE
                    hTh = fpool.tile([128, NF, 512], BF16, tag="hTh")
                    for mg in range(NF // 4):
                        w1g = w1pool.tile([128, ND, 512], BF16, tag="w1g")
                        nc.sync.dma_start(
                            out=w1g,
                            in_=w1_d.ap()[:, :, mg * 512:(mg + 1) * 512]
                            .rearrange("c p f -> p c f"))
                        for mi in range(4):
                            m = mg * 4 + mi
                            ps = psB.tile([128, 512], F32, tag="B")
                            for c in range(ND):
                                nc.tensor.matmul(
                                    ps, w1g[:, c, ts(mi, 128)],
                                    z2T[:, c, tbs],
                                    start=(c == 0), stop=(c == ND - 1))
                            nc.vector.tensor_scalar(
                                out=hTh[:, m, :], in0=ps,
                                scalar1=b1t[:, m:m + 1], scalar2=0.0,
                                op0=ALU.add, op1=ALU.max)
                    # FFN2 half + residual -> out
                    for db in range(2):
                        w2cs = []
                        for cg in range(4):
                            w2c = w2pool.tile([128, 8, 512], BF16, tag="w2c")
                            nc.sync.dma_start(
                                out=w2c,
                                in_=w2_d.ap()[cg * 8:(cg + 1) * 8, :,
                                              db * 512:(db + 1) * 512]
                                .rearrange("c p f -> p c f"))
                            w2cs.append(w2c)
                        for tl in range(4):
                            tt = tb * 4 + tl
                            ps = psB.tile([128, 512], F32, tag="B")
                            for c in range(NF):
                                nc.tensor.matmul(ps, hTh[:, c, ts(tl, 128)],
                                                 w2cs[c // 8][:, c % 8, :],
                                                 start=(c == 0), stop=False)
                            o = 3 * D + db * 512
                            nc.tensor.matmul(ps, onesrow, brows[0:1, o:o + 512],
                                             start=False, stop=True)
                            ot = opool.tile([128, 512], F32, tag="ot")
                            nc.vector.tensor_add(
                                out=ot, in0=ps,
                                in1=x2[:, tt, db * 512:(db + 1) * 512])
                            nc.sync.dma_start(
                                out=outr[:, tt, db * 512:(db + 1) * 512],
                                in_=ot)

    nc.compile()
    return nc


def _prep_inputs(inputs):
    """Host-side preprocessing: fold LN affine, permute rope cols, cast bf16."""
    f32 = np.float32
    x = np.asarray(inputs["x"], f32)
    wq = np.asarray(inputs["wq"], f32)
    wk = np.asarray(inputs["wk"], f32)
    wv = np.asarray(inputs["wv"], f32)
    w_proj = np.asarray(inputs["w_proj"], f32)
    b_proj = np.asarray(inputs["b_proj"], f32)
    ln1_w = np.asarray(inputs["ln1_w"], f32)
    ln1_b = np.asarray(inputs["ln1_b"], f32)
    ln2_w = np.asarray(inputs["ln2_w"], f32)
    ln2_b = np.asarray(inputs["ln2_b"], f32)
    w1 = np.asarray(inputs["w1"], f32)
    b1 = np.asarray(inputs["b1"], f32)
    w2 = np.asarray(inputs["w2"], f32)
    b2 = np.asarray(inputs["b2"], f32)

    bf = ml_dtypes.bfloat16
    perm = np.concatenate([np.arange(0, HS, 2), np.arange(1, HS, 2)])
    idx = (np.arange(H)[:, None] * HS + perm[None, :]).reshape(-1)

    wq_flat = wq.transpose(1, 0, 2).reshape(D, H * HS)
    wk_flat = wk.transpose(1, 0, 2).reshape(D, H * HS)
    wv_flat = wv.transpose(1, 0, 2).reshape(D, H * HS)
    wq_p = wq_flat[:, idx]
    wk_p = wk_flat[:, idx]

    def wlayout(w):  # [D, D] -> [128, ND, D]  (p = d_in, c = d_chunk)
        return np.ascontiguousarray(
            w.reshape(ND, 128, D).transpose(1, 0, 2)).astype(bf)

    wq_h = wlayout(ln1_w[:, None] * wq_p)
    wk_h = wlayout(ln1_w[:, None] * wk_p)
    wv_h = wlayout(ln1_w[:, None] * wv_flat)
    wp_h = wlayout(w_proj)
    w1_h = np.ascontiguousarray(
        (ln2_w[:, None] * w1).reshape(ND, 128, F)).astype(bf)
    w2_h = np.ascontiguousarray(w2.reshape(NF, 128, D)).astype(bf)

    bq = ln1_b @ wq_p
    bk = ln1_b @ wk_p
    bv = ln1_b @ wv_flat
    bproj_eff = b_proj + bv @ w_proj
    b1_eff = ln2_b @ w1 + b1
    brows = np.concatenate([bq, bk, bproj_eff, b2]).reshape(1, 4 * D).astype(bf)
    b1t = np.ascontiguousarray(b1_eff.reshape(NF, 128).T).astype(f32)

    # rope tables: [128, NT, 4, HS]; 4 = (cos_q, sin_q, cos_k, sin_k)
    t = np.arange(T, dtype=f32)
    th = (1.0 / 10000.0 ** (np.arange(0, HS, 2, dtype=f32) / f32(HS))).astype(f32)
    ang = t[:, None] * th[None, :]
    cos = np.concatenate([np.cos(ang), np.cos(ang)], 1)           # [T, HS]
    sin = np.concatenate([-np.sin(ang), np.sin(ang)], 1)
    sc = f32(HS) ** f32(-0.5)
    rope = np.stack([cos * sc, sin * sc, cos, sin], 1)            # [T, 4, HS]
    rope_h = np.ascontiguousarray(
        rope.reshape(NT, 128, 4, HS).transpose(1, 0, 2, 3)).astype(bf)

    # causal 0/1 pair-masks: pair 0 = s-tiles (j=0, j=1), pair 1 = (j=2, j=3)
    sl = np.arange(128)[:, None]
    ql = np.arange(512)[None, :]
    m4 = [(j * 128 + sl <= ql).astype(bf) for j in range(4)]
    mask_h = np.stack([np.concatenate([m4[0], m4[1]], 1),
                       np.concatenate([m4[2], m4[3]], 1)])        # [2, 128, 1024]
    mask_h = np.ascontiguousarray(mask_h.transpose(1, 0, 2))      # [128, 2, 1024]

    common = {
        "wq": wq_h, "wk": wk_h, "wv": wv_h, "wp": wp_h,
        "w1": w1_h, "w2": w2_h,
        "rope": rope_h, "mask": mask_h,
        "ones64": np.ones((128, 64), bf),
        "onesrow": np.ones((1, 128), bf),
        "brows": brows, "b1t": b1t,
    }
    in_maps = [dict(common, x=np.ascontiguousarray(x[b])) for b in range(B)]
    return in_maps


_NC_CACHE = {}


def get_nc():
    if "nc" not in _NC_CACHE:
        _NC_CACHE["nc"] = build_kernel()
    return _NC_CACHE["nc"]


def kernel(**inputs):
    nc = get_nc()
    in_maps = _prep_inputs(inputs)
    res = run_bass_kernel_spmd(nc, in_maps, core_ids=list(range(NCORES)))
    out = np.stack([res.results[i]["out"] for i in range(NCORES)])
    return out.astype(np.float32)
